# revision 65
# baseline (speedup 1.0000x reference)
"""MultiHeadDiffAttention Trainium2 kernel.

Strategy (8 NeuronCores, SPMD):
  - Shard: batch (B=2) x head-groups (16 heads -> 4 groups of 4).
    Core c handles b = c//4, heads 4*(c%4) .. 4*(c%4)+3.
  - Differential attention folded into one 128-dim attention per head:
      q' = [q1 * scale | q2 * (-lam*scale)],  k' = [k1 | k2]
  - QKV projection runs in fp8(e4m3) DoubleRow with an hi/lo error-
    compensated 3-term expansion:
      x @ W ~= x_hi@W_hi + x_lo@W_hi + x_hi@W_lo
    where x_hi = fp8(x*sx), x_lo = fp8(x*sx - x_hi) (the lo part uses
    fp8's exponent range, so all terms share one scale and accumulate
    in the same PSUM group).  24 DoubleRow passes (3 terms x 8 e-pairs)
    at 0.5 cyc/row replace 16 bf16 passes at 1.0 -> 0.75x PE cost with
    ~1e-3 relative error (bf16-comparable).
  - Attention (logits, exp, O=V^T P^T) in bf16 with f32 PSUM; exp on
    ScalarE over 1024-wide PSUM pairs with the fp8 scale compensation
    folded into the activation's scale operand.  The softmax
    denominator is computed OFF the PE: a bf16 pair tree on DVE + f32
    combine, finished by a gpsimd partition-allreduce whose output is
    already broadcast across partitions.
  - The exp stream paces attention (ACT is the slowest engine there),
    so every dense-PE block that can move is interleaved into the
    exp-paced gaps at odd-s granularity: the second t-block's Q
    projection fills attention block 0; the out-projection of block 0
    fills attention block 1; block 1's projection runs as a dense
    tail on the then-idle psa PSUM slots with ACT doing its copies.
  - Out-projection in bf16; output f32; host sums the 4 head-group
    partials per batch element.

All data-dependent scale factors are powers of two, folded host-side
into the weights / the exp scale input, so the compiled program is
input-independent.
"""

import contextlib
import math

import numpy as np

B, T, E = 2, 2048, 2048
N_HEAD = 16
HD = 64                       # per-component head dim (q1/k1/q2/k2)
DV = 128                      # v head dim
SCALE = HD ** -0.5
LAMBDA_INIT = 0.8 - 0.6 * math.exp(-0.3 * (1 - 1))
P = 128
NHC = 4                       # heads per core
CQ = NHC * DV                 # 512: per-core q'/k'/v width
N_CORES = 8
NE = E // P                   # 16 contraction chunks
NEP = NE // 2                 # 8 DoubleRow e-pairs
NS = T // P                   # 16 s chunks

_NC_CACHE = None
DEBUG_DUMP = False            # set True to add intermediate-tensor outputs


def _build_nc():
    import concourse.mybir as mybir
    import concourse.tile as tile
    from concourse import bacc
    from concourse import bass_isa

    RADD = bass_isa.ReduceOp.add
    f32 = mybir.dt.float32
    bf16 = mybir.dt.bfloat16
    f8 = mybir.dt.float8e4
    EXP = mybir.ActivationFunctionType.Exp
    DR = mybir.MatmulPerfMode.DoubleRow

    nc = bacc.Bacc("TRN2", target_bir_lowering=False, debug=False,
                   num_devices=N_CORES)
    # all inputs arrive pre-laid-out in SBUF order (partition-major), so
    # every input DMA is a plain low-descriptor-count contiguous copy
    xh = nc.dram_tensor("xh", [P, NEP, 2, T], f8, kind="ExternalInput").ap()
    xl = nc.dram_tensor("xl", [P, NEP, 2, T], f8, kind="ExternalInput").ap()
    wqh = nc.dram_tensor("wqh", [P, NE, CQ], f8, kind="ExternalInput").ap()
    wql = nc.dram_tensor("wql", [P, NE, CQ], f8, kind="ExternalInput").ap()
    wkh = nc.dram_tensor("wkh", [P, NE, CQ], f8, kind="ExternalInput").ap()
    wkl = nc.dram_tensor("wkl", [P, NE, CQ], f8, kind="ExternalInput").ap()
    wvh = nc.dram_tensor("wvh", [P, NE, CQ], f8, kind="ExternalInput").ap()
    wvl = nc.dram_tensor("wvl", [P, NE, CQ], f8, kind="ExternalInput").ap()
    wph = nc.dram_tensor("wph", [P, NHC, E], f8, kind="ExternalInput").ap()
    wpl = nc.dram_tensor("wpl", [P, NHC, E], f8, kind="ExternalInput").ap()
    cs = nc.dram_tensor("cs", [P, 1], f32, kind="ExternalInput").ap()
    osc = nc.dram_tensor("osc", [P, 1], f32, kind="ExternalInput").ap()
    psc = nc.dram_tensor("psc", [P, 1], f32, kind="ExternalInput").ap()
    out = nc.dram_tensor("out", [T, E], bf16, kind="ExternalOutput").ap()
    dbg = {}
    if DEBUG_DUMP:
        for nm, shp in (("d_kt", [P, NHC, T]), ("d_qt0", [P, NHC, 1024]),
                        ("d_qt1", [P, NHC, 1024]), ("d_vsb", [P, NS, CQ]),
                        ("d_ot0", [P, NHC, 1024]), ("d_ot1", [P, NHC, 1024])):
            dbg[nm] = nc.dram_tensor(nm, shp, bf16,
                                     kind="ExternalOutput").ap()

    with tile.TileContext(nc) as tc:
        with tc.tile_pool(name="res", bufs=1) as res:
            # qt/ot are split per t-1024 block: the gap-filler chunks
            # write block 1 while attention reads block 0 (and vice versa
            # for ot), and tile-granular dependency tracking must not
            # serialize those against each other.
            qt = [res.tile([P, NHC, 1024], bf16, name=f"qt{i}")
                  for i in range(2)]                        # Q'^T [c, h, t]
            kt = res.tile([P, NHC, T], bf16, name="kt")     # K'^T [c, h, s]
            vsb = res.tile([P, NS, CQ], bf16, name="vsb")   # V [s%128, sc, dv]
            oth = [res.tile([P, NHC, 1024], f8, name=f"oth{i}")
                   for i in range(2)]                       # O^T hi [dv,h,t]
            otl = [res.tile([P, NHC, 1024], f8, name=f"otl{i}")
                   for i in range(2)]                       # O^T lo
            wpth = res.tile([P, NHC, E], f8, name="wpth")   # W_proj hi
            wptl = res.tile([P, NHC, E], f8, name="wptl")   # W_proj lo
            cst = res.tile([P, 1], f32, name="cst")
            osct = res.tile([P, 1], f32, name="osct")
            psct = res.tile([P, 1], f32, name="psct")
            # (wpt DMA issued after the K rounds, overlapping V/Q)

            def att_head(pe, pm, pps, bufs, ti2, h, filler=None):
                """Attention for one (t-1024 block, head).

                `filler(k)` emits a dense-PE chunk after the k-th odd-s
                iteration to fill the exp-paced gaps (PE issues in
                order, so gaps must be filled at emission time).
                Logits/exp run a depth-2 modulo schedule ahead of the
                consuming O-matmuls so PE never waits on ACT.
                """
                qtb = qt[ti2]
                pso = [pps.tile([P, 512], f32, name=f"pso{half}",
                                tag="pso", bufs=bufs["pso"])
                       for half in range(2)]
                # Z = col-sums of exp tiles: bf16 pair tree on DVE + f32
                # combine + gpsimd partition-allreduce (pre-broadcast
                # output) — no PE cycles on the softmax denominator.
                etp = [None] * 8
                etq = [None] * 4
                ets = [None] * NS

                def consume(s):
                    et = ets[s]
                    for half in range(2):
                        nc.tensor.matmul(
                            pso[half],
                            lhsT=vsb[:, s, h * P:(h + 1) * P],
                            rhs=et[:, half * 512:(half + 1) * 512],
                            start=(s == 0), stop=(s == NS - 1),
                        )
                    if s % 2 == 0:
                        etp[s // 2] = et
                    else:
                        j = s // 2
                        pj = pe.tile([P, 1024], bf16, name="etp",
                                     tag="etp", bufs=bufs["etp"])
                        nc.vector.tensor_add(pj, etp[j], et)
                        etp[j] = pj
                        if s % 4 == 3:
                            # quad-level adds run on the otherwise-idle
                            # gpsimd engine to keep DVE off the critical
                            # path (DVE carries normalization + copies)
                            i = s // 4
                            qi = pe.tile([P, 1024], bf16, name="etq",
                                         tag="etq", bufs=4)
                            nc.gpsimd.tensor_add(
                                qi, etp[i * 2], etp[i * 2 + 1])
                            etq[i] = qi
                        if filler is not None:
                            filler(s // 2)

                for s in range(NS):
                    et = pe.tile([P, 1024], bf16, name="et",
                                 tag="et", bufs=bufs["et"])
                    ets[s] = et
                    psa = pps.tile([P, 1024], f32, name="psa",
                                   tag="psa", bufs=bufs["psa"])
                    for half in range(2):
                        nc.tensor.matmul(
                            psa[:, half * 512:(half + 1) * 512],
                            lhsT=kt[:, h, s * P:(s + 1) * P],
                            rhs=qtb[:, h, half * 512:
                                    (half + 1) * 512],
                            start=True, stop=True,
                        )
                    nc.scalar.activation(et, psa, EXP, scale=cst[:, 0:1])
                    if s >= 2:
                        consume(s - 2)
                consume(NS - 2)
                consume(NS - 1)
                za = pm.tile([P, 1024], f32, name="za", tag="za",
                             bufs=bufs["z"])
                zb = pm.tile([P, 1024], f32, name="zb", tag="zb",
                             bufs=bufs["z"])
                nc.vector.tensor_add(za, etq[0], etq[1])
                nc.vector.tensor_add(zb, etq[2], etq[3])
                zs = pm.tile([P, 1024], f32, name="zs", tag="zs",
                             bufs=bufs["z"])
                nc.vector.tensor_add(zs, za, zb)
                zr = pm.tile([P, 1024], f32, name="zr", tag="zr",
                             bufs=bufs["z"])
                nc.gpsimd.partition_all_reduce(
                    zr, zs, channels=P, reduce_op=RADD)
                rb = pm.tile([P, 1024], f32, name="rb", tag="rb",
                             bufs=bufs["rb"])
                nc.vector.reciprocal(rb, zr)
                for half in range(2):
                    # ot in fp8 hi/lo: tmp = O/Z (f32), hi = fp8(tmp*osc)
                    # on ACT, lo = fp8(tmp*osc - hi) in one DVE stt op
                    sl = slice(half * 512, (half + 1) * 512)
                    tmp = pm.tile([P, 512], f32, name="tmp", tag="tmp",
                                  bufs=2)
                    nc.vector.tensor_mul(tmp, pso[half], rb[:, sl])
                    nc.vector.tensor_scalar_mul(
                        oth[ti2][:, h, sl], tmp, osct[:, 0:1])
                    nc.vector.scalar_tensor_tensor(
                        otl[ti2][:, h, sl], tmp, osct[:, 0:1],
                        oth[ti2][:, h, sl],
                        op0=mybir.AluOpType.mult,
                        op1=mybir.AluOpType.subtract)

            # ---------- Phase A: QKV projections (fp8 hi/lo DoubleRow) ----
            # x and the q-weights live through attention block 0, whose
            # exp-paced gaps the second Q-projection round fills.
            with (
                tc.tile_pool(name="pa_x", bufs=1) as pa_x,
                tc.tile_pool(name="pa_wq", bufs=1) as pa_wq,
            ):
                # one tile per e-pair, each filled by a single whole-
                # tile DMA: DMA writes into slices of a shared tile rely
                # on sub-tile dependency tracking, which raced (NaNs)
                xsb = {p: [pa_x.tile([P, 2, T], f8, name=f"x{p}{j}")
                           for j in range(NEP)] for p in ("h", "l")}
                xsrc = {"h": xh, "l": xl}
                xloaded = [False] * NEP
                wsrc = {"kh": wkh, "kl": wkl, "vh": wvh,
                        "vl": wvl, "qh": wqh, "ql": wql}
                wsb = {}

                def need_x(j):
                    # JIT per-e-pair loads: the first K-round matmuls
                    # start as soon as e-pair 0 lands, not after 8MB.
                    if not xloaded[j]:
                        for part in ("h", "l"):
                            nc.sync.dma_start(xsb[part][j],
                                              xsrc[part][:, j, :, :])
                        xloaded[j] = True

                def prefetch_first():
                    need_x(0)
                    need_x(1)

                def qk_passes(ps8, wn, rhs_of, j, first, bank_n=8,
                              c_off=0):
                    for ti, (wt, xt) in enumerate(
                            ((wsb[wn + "h"], xsb["h"][j]),
                             (wsb[wn + "h"], xsb["l"][j]),
                             (wsb[wn + "l"], xsb["h"][j]))):
                        for b in range(bank_n):
                            c = c_off + b // 2
                            nc.tensor.matmul(
                                ps8[b],
                                lhsT=wt[:, 2 * j:2 * j + 2,
                                        c * P:(c + 1) * P],
                                rhs=xt[:, :, rhs_of(b)],
                                start=(j == 0 and ti == 0 and first),
                                stop=(j == NEP - 1 and ti == 2),
                                perf_mode=DR,
                            )

                with (
                    tc.tile_pool(name="pa_wkv", bufs=1) as pa_wkv,
                    tc.tile_pool(name="pa_ps", bufs=1,
                                 space="PSUM") as pa_ps,
                ):
                    def need_w(*names):
                        # weight DMA issued just before the round using
                        # it (K first, with the leading x chunks)
                        for nm in names:
                            if nm not in wsb:
                                pool = pa_wq if nm[0] == "q" else pa_wkv
                                t_ = pool.tile([P, NE, CQ], f8,
                                               name=f"w{nm}")
                                if nm[0] == "k":
                                    # quarters: the j=0 matmuls start
                                    # after 0.25MB instead of 1MB
                                    for q4 in range(4):
                                        nc.sync.dma_start(
                                            t_[:, q4 * 4:(q4 + 1) * 4, :],
                                            wsrc[nm][:, q4 * 4:(q4 + 1) * 4,
                                                     :])
                                else:
                                    nc.sync.dma_start(t_, wsrc[nm])
                                wsb[nm] = t_

                    def qk_round(wn, dst, bo, dst_off):
                        # dst_off: kt is a full-T tile (offset by block);
                        # qt[0] is a per-block tile (offset 0).
                        # Two 4-bank sub-rounds: the first half's copies
                        # overlap the second half's matmuls, so round
                        # transitions never stall on bank turnover.
                        t0 = bo * 1024
                        need_x(0)
                        need_w(wn + "h", wn + "l")
                        for sub in range(2):
                            pss = [pa_ps.tile([P, 512], f32, name="psqk",
                                              tag="pa_ps", bufs=8)
                                   for _ in range(4)]
                            for j in range(NEP):
                                need_x(j)
                                qk_passes(
                                    pss, wn,
                                    lambda b: slice(t0 + (b % 2) * 512,
                                                    t0 + (b % 2 + 1) * 512),
                                    j, True, bank_n=4, c_off=sub * 2)
                            for b in range(4):
                                c, half = sub * 2 + b // 2, b % 2
                                dsl = dst[:, c, dst_off + half * 512:
                                          dst_off + (half + 1) * 512]
                                if b % 2 == 0:
                                    nc.scalar.copy(dsl, pss[b])
                                else:
                                    nc.vector.tensor_copy(dsl, pss[b])

                    def v_round(bo):
                        need_w("vh", "vl")
                        for sub in range(2):
                            pss = [pa_ps.tile([P, 512], f32, name="psv",
                                              tag="pa_ps", bufs=8)
                                   for _ in range(4)]
                            for j in range(NEP):
                                need_x(j)
                                for ti, (xt, wt) in enumerate(
                                        ((xsb["h"][j], wsb["vh"]),
                                         (xsb["l"][j], wsb["vh"]),
                                         (xsb["h"][j], wsb["vl"]))):
                                    for b in range(4):
                                        tc_ = bo * 8 + sub * 4 + b
                                        nc.tensor.matmul(
                                            pss[b],
                                            lhsT=xt[:, :,
                                                    tc_ * P:(tc_ + 1) * P],
                                            rhs=wt[:, 2 * j:2 * j + 2, :],
                                            start=(j == 0 and ti == 0),
                                            stop=(j == NEP - 1 and ti == 2),
                                            perf_mode=DR,
                                        )
                            for b in range(4):
                                tc_ = bo * 8 + sub * 4 + b
                                if b % 2 == 0:
                                    nc.scalar.copy(vsb[:, tc_, :], pss[b])
                                else:
                                    nc.vector.tensor_copy(
                                        vsb[:, tc_, :], pss[b])

                    for bo in range(2):
                        qk_round("k", kt, bo, bo * 1024)
                    nc.sync.dma_start(cst, cs)
                    nc.sync.dma_start(osct, osc)
                    nc.sync.dma_start(psct, psc)
                    nc.sync.dma_start(wpth, wph)
                    nc.sync.dma_start(wptl, wpl)
                    for bo in range(2):
                        v_round(bo)
                    qk_round("q", qt[0], 0, 0)

                # ---- attention block 0, Q(bo1) round as gap filler ----
                # the attention PSUM pool spans BOTH attention scopes (via
                # ExitStack): B2's first psa alloc then waits only on slot
                # cycling, not on a scope handover that drains behind
                # att(0,h3)'s whole Z-tail.
                att_es = contextlib.ExitStack()
                pb_ps1 = att_es.enter_context(
                    tc.tile_pool(name="pb_ps", bufs=1, space="PSUM"))
                with (
                    tc.tile_pool(name="pb_e1", bufs=1) as pb_e1,
                    tc.tile_pool(name="pb_m1", bufs=1) as pb_m1,
                ):
                    def q1_chunk(i):
                        # one (c, half) stripe of the bo=1 Q projection:
                        # 24 dense DoubleRow passes + a copy
                        c, half = i // 2, i % 2
                        ps = pb_ps1.tile([P, 512], f32, name="qf",
                                         tag="qf", bufs=1)
                        n = 0
                        for j in range(NEP):
                            for wt, xt in ((wsb["qh"], xsb["h"][j]),
                                           (wsb["qh"], xsb["l"][j]),
                                           (wsb["ql"], xsb["h"][j])):
                                nc.tensor.matmul(
                                    ps,
                                    lhsT=wt[:, 2 * j:2 * j + 2,
                                            c * P:(c + 1) * P],
                                    rhs=xt[:, :,
                                           1024 + half * 512:
                                           1024 + (half + 1) * 512],
                                    start=(n == 0), stop=(n == 3 * NEP - 1),
                                    perf_mode=DR,
                                )
                                n += 1
                        nc.vector.tensor_copy(
                            qt[1][:, c, half * 512:(half + 1) * 512], ps)

                    bufs1 = {"et": 4, "etp": 3, "z": 1, "rb": 1,
                             "psa": 2, "pso": 3}

                    def qfill(h, k):
                        g = h * 8 + k
                        if g % 4 == 2:
                            q1_chunk(g // 4)

                    for h in range(NHC):
                        att_head(pb_e1, pb_m1, pb_ps1, bufs1, 0, h,
                                 filler=lambda k, hh=h: qfill(hh, k))

            # ---- attention block 1 + out-projection ----
            pb_ps2 = pb_ps1
            with (
                tc.tile_pool(name="pb_e2", bufs=1) as pb_e2,
                tc.tile_pool(name="pb_m2", bufs=1) as pb_m2,
                tc.tile_pool(name="pd", bufs=1) as pd,
            ):
                osb_of = {}

                def proj_chunk(ti2, eo, tj, tag="qf", bufs=1,
                               copy_eng=None):
                    # one [t128, e512] tile of the out-projection.
                    # Interleaved chunks use the spare "psd" bank;
                    # dense-tail chunks borrow the then-idle psa slots
                    # and the exp-free ACT engine for their copies.
                    if tj == 0:
                        osb_of[(ti2, eo)] = pd.tile(
                            [P, 8, 512], bf16, name="osb", tag="osb",
                            bufs=3)
                    osb = osb_of[(ti2, eo)]
                    ps = pb_ps2.tile([P, 512], f32, name="psd",
                                     tag=tag, bufs=bufs)
                    n = 0
                    for ohh, wpp in ((oth[ti2], wpth), (otl[ti2], wpth),
                                     (oth[ti2], wptl)):
                        for i in range(2):
                            nc.tensor.matmul(
                                ps,
                                lhsT=ohh[:, 2 * i:2 * i + 2,
                                         tj * P:(tj + 1) * P],
                                rhs=wpp[:, 2 * i:2 * i + 2,
                                        eo * 512:(eo + 1) * 512],
                                start=(n == 0), stop=(n == 5),
                                perf_mode=DR,
                            )
                            n += 1
                    if copy_eng == "act" or (copy_eng is None
                                              and tj % 2 == 0):
                        nc.scalar.activation(
                            osb[:, tj, :], ps,
                            mybir.ActivationFunctionType.Copy,
                            scale=psct[:, 0:1])
                    else:
                        nc.vector.tensor_scalar_mul(osb[:, tj, :], ps,
                                                    psct[:, 0:1])
                    if tj in (3, 7):
                        hb = tj // 4
                        nc.sync.dma_start(
                            out[ti2 * 1024 + hb * 512:
                                ti2 * 1024 + (hb + 1) * 512,
                                eo * 512:(eo + 1) * 512].rearrange(
                                    "(tj p) e -> p tj e", p=P),
                            osb[:, hb * 4:(hb + 1) * 4, :])

                bufs2 = {"et": 8, "etp": 4, "z": 2, "rb": 2,
                         "psa": 2, "pso": 3}
                for h in range(NHC):
                    att_head(pb_e2, pb_m2, pb_ps2, bufs2, 1, h,
                             filler=lambda k, eo=h: proj_chunk(0, eo, k))
                # last block's projection has no attention to hide in
                for eo in range(4):
                    for tj in range(8):
                        proj_chunk(1, eo, tj, tag="psa", bufs=2)
                if DEBUG_DUMP:
                    for nm, t_ in (("d_kt", kt), ("d_qt0", qt[0]),
                                   ("d_qt1", qt[1]), ("d_vsb", vsb)):
                        nc.sync.dma_start(dbg[nm], t_)
            att_es.close()

    nc.compile()
    return nc


def _get_nc():
    global _NC_CACHE
    if _NC_CACHE is None:
        _NC_CACHE = _build_nc()
    return _NC_CACHE


def _pow2_scale(a):
    """Largest power of two s with absmax(a)*s <= 220.

    ml_dtypes.float8_e4m3 (what mybir.dt.float8e4 maps to) is the IEEE
    variant whose max finite value is 240 — not e4m3fn's 448.
    """
    m = float(np.abs(a).max())
    if m == 0.0:
        return 1.0
    return 2.0 ** math.floor(math.log2(220.0 / m))


def _hilo(a, s):
    import ml_dtypes
    f8 = ml_dtypes.float8_e4m3
    hi = (a * s).astype(f8)
    lo = (a * s - hi.astype(np.float32)).astype(f8)
    return hi, lo


def _sbuf_layout_w(w):
    """[E, cols] -> [P, NE, cols] partition-major (SBUF order)."""
    return np.ascontiguousarray(
        w.reshape(NE, P, w.shape[1]).transpose(1, 0, 2))


def _sbuf_layout_x(x):
    """[E, T] -> [P, NEP, 2, T] partition-major e-pair tiles."""
    return np.ascontiguousarray(
        x.reshape(NEP, 2, P, T).transpose(2, 0, 1, 3))


def _shard_inputs(x, W_attn, W_proj, lambda_q1, lambda_k1,
                  lambda_q2, lambda_k2):
    import ml_dtypes
    bf16 = ml_dtypes.bfloat16
    x = np.asarray(x, np.float32)
    W_attn = np.asarray(W_attn, np.float32)
    W_proj = np.asarray(W_proj, np.float32)
    lam = float(np.exp(np.dot(np.asarray(lambda_q1, np.float32),
                              np.asarray(lambda_k1, np.float32)))
                - np.exp(np.dot(np.asarray(lambda_q2, np.float32),
                                np.asarray(lambda_k2, np.float32)))
                + LAMBDA_INIT)
    Cb = E // 2  # 1024: q1/k1/q2/k2 block width in W_attn
    in_maps = []
    for c in range(N_CORES):
        b, hg = divmod(c, 4)
        heads = [4 * hg + j for j in range(NHC)]
        wq_c = np.empty((E, CQ), np.float32)
        wk_c = np.empty((E, CQ), np.float32)
        wv_c = np.empty((E, CQ), np.float32)
        wp_c = np.empty((CQ, E), np.float32)
        for j, h in enumerate(heads):
            wq_c[:, j * P:j * P + HD] = W_attn[:, h * HD:(h + 1) * HD] * SCALE
            wq_c[:, j * P + HD:(j + 1) * P] = (
                W_attn[:, 2 * Cb + h * HD:2 * Cb + (h + 1) * HD]
                * (-lam * SCALE))
            wk_c[:, j * P:j * P + HD] = W_attn[:, Cb + h * HD:Cb + (h + 1) * HD]
            wk_c[:, j * P + HD:(j + 1) * P] = (
                W_attn[:, 3 * Cb + h * HD:3 * Cb + (h + 1) * HD])
            wv_c[:, j * P:(j + 1) * P] = (
                W_attn[:, 4 * Cb + h * DV:4 * Cb + (h + 1) * DV])
            wp_c[j * P:(j + 1) * P, :] = (
                W_proj[h * DV:(h + 1) * DV, :] * (1.0 - LAMBDA_INIT))
        xT = np.ascontiguousarray(x[b].T)
        sx = _pow2_scale(xT)
        swq = _pow2_scale(wq_c)
        swk = _pow2_scale(wk_c)
        swv = _pow2_scale(wv_c)
        swp = _pow2_scale(wp_c)
        sot = 32.0
        xh_, xl_ = _hilo(xT, sx)
        wqh_, wql_ = _hilo(wq_c, swq)
        wkh_, wkl_ = _hilo(wk_c, swk)
        wvh_, wvl_ = _hilo(wv_c, swv)
        wph_, wpl_ = _hilo(wp_c, swp)

        def wp_layout(a):
            return np.ascontiguousarray(
                a.reshape(NHC, P, E).transpose(1, 0, 2))

        in_maps.append({
            "xh": _sbuf_layout_x(xh_), "xl": _sbuf_layout_x(xl_),
            "wqh": _sbuf_layout_w(wqh_), "wql": _sbuf_layout_w(wql_),
            "wkh": _sbuf_layout_w(wkh_), "wkl": _sbuf_layout_w(wkl_),
            "wvh": _sbuf_layout_w(wvh_), "wvl": _sbuf_layout_w(wvl_),
            "wph": wp_layout(wph_), "wpl": wp_layout(wpl_),
            "cs": np.full((P, 1), 1.0 / (sx * sx * swq * swk), np.float32),
            "osc": np.full((P, 1), sot / (sx * swv), np.float32),
            "psc": np.full((P, 1), 1.0 / (sot * swp), np.float32),
        })
    return in_maps


def _run(inputs, trace=False):
    from concourse.bass_utils import run_bass_kernel_spmd
    nc = _get_nc()
    in_maps = _shard_inputs(**inputs)
    res = run_bass_kernel_spmd(nc, in_maps, list(range(N_CORES)),
                               trace=trace)
    out = np.zeros((B, T, E), np.float32)
    for c in range(N_CORES):
        out[c // 4] += np.asarray(res.results[c]["out"]).astype(np.float32)
    return out, res


def kernel(x, W_attn, W_proj, lambda_q1, lambda_k1, lambda_q2, lambda_k2):
    out, _ = _run(dict(x=x, W_attn=W_attn, W_proj=W_proj,
                       lambda_q1=lambda_q1, lambda_k1=lambda_k1,
                       lambda_q2=lambda_q2, lambda_k2=lambda_k2))
    return out
